# revision 15
# baseline (speedup 1.0000x reference)
"""Multi-head attention (B=2, S=2048, E=768, H=12, D=64) on 8 NeuronCores.

Sharding: core c -> batch b = c//4, head group hg = c%4 (3 heads each).
Each core computes the qkv projection for its 3 heads, attention, and a
partial output projection (rows of w_proj for its heads). Host sums the 4
partials per batch and adds the bias terms (tensor-parallel unshard).

Device dataflow (everything transposed so no on-chip transposes are needed,
and every matmul has a full K=128 contraction — K=64 matmuls run the PE at
half clock):
  xT [768, 2048]  (host-pretransposed, fp16)
  qkT[t] = (w_qk_tile_t)^T @ xT -> [128, 2048] tiles t=0..4 with w columns
           laid out [q0|q1], [k0|0], [0|k1], [0|q2], [0|k2]: each head's
           scoresT matmul then uses a full-128-partition stationary whose
           zero half kills the other head's rows.
  v'   = xT_tile^T @ w_v -> per-head per-Sk-block [128, 128] blocks:
           cols 0:64 = v, cols 64:128 = 1.0, so the AV matmul also produces
           the softmax denominator replicated across partitions 64:128.
  pT   = exp(scoresT / 8)   (ScalarE, PSUM -> SBUF fp16)
  avT  = v'^T @ pT          -> [128, 512] PSUM per (head, Sq-chunk);
           rows 64:128 = denominator l
  outT = avT[0:64] * approx(1/l)   (DVE reciprocal_approx_fast, ~51 ULP)
  yT  += w_proj_part^T @ outT      -> [768, 2048] fp32 partial, DMA'd out.

Emission order keeps ScalarE (the 104us exp stream paces the middle of the
kernel) fed from ~15us on, and each head's AV matmuls are emitted after the
next head's scores so the Tile scheduler uses them to fill TensorE gaps.
"""

import threading

import numpy as np

import concourse.bass as bass
import concourse.tile as tile
from concourse import bacc, mybir
from concourse.bass import ts, ds
from concourse.bass_utils import run_bass_kernel_spmd

F32 = mybir.dt.float32
F16 = mybir.dt.float16

EMBED = 768
NH = 12
D = 64
B = 2
S = 2048
HPC = 3          # heads per core
NCORES = 8
P = 128
KC = EMBED // P  # 6 contraction chunks
NQ = S // 512    # 4 Sq chunks of 512
NSK = S // P     # 16 Sk blocks
NT = 5           # qk projection tiles


def _build_kernel(nc):
    xT = nc.dram_tensor("xT", [EMBED, S], F16, kind="ExternalInput").ap()
    wqk = nc.dram_tensor("w_qk", [EMBED, 3 * P], F16, kind="ExternalInput").ap()
    bqk = nc.dram_tensor("b_qk", [NT * P], F32, kind="ExternalInput").ap()
    wv = nc.dram_tensor("w_v", [EMBED, HPC * D], F16, kind="ExternalInput").ap()
    wp = nc.dram_tensor("w_p", [2 * P, EMBED], F16, kind="ExternalInput").ap()
    yT = nc.dram_tensor("yT", [EMBED, S], F32, kind="ExternalOutput").ap()

    with tile.TileContext(nc) as tc:
        with (
            tc.tile_pool(name="wpool", bufs=1) as wpool,
            tc.tile_pool(name="xpool", bufs=1) as xpool,
            tc.tile_pool(name="qkpool", bufs=1) as qkpool,
            tc.tile_pool(name="vpool", bufs=1) as vpool,
            tc.tile_pool(name="ptpool", bufs=26) as ptpool,
            tc.tile_pool(name="opool", bufs=1) as opool,
            tc.tile_pool(name="rlpool", bufs=4) as rlpool,
            tc.tile_pool(name="psum", bufs=2, space="PSUM") as psum,
        ):
            # ---- loads (xT interleaved with wqk so qk matmuls start early)
            wqk_t = []
            xT_t = []
            for k in range(KC):
                xT_k = xpool.tile([P, S], F16, name=f"xT{k}")
                nc.sync.dma_start(out=xT_k, in_=xT[ts(k, P), :])
                xT_t.append(xT_k)
                wqk_k = wpool.tile([P, 3 * P], F16, name=f"wqk{k}")
                nc.sync.dma_start(out=wqk_k, in_=wqk[ts(k, P), :])
                wqk_t.append(wqk_k)
            bq_t = []
            for t in range(NT):
                bq_k = wpool.tile([P, 1], F32, name=f"bq{t}")
                nc.sync.dma_start(
                    out=bq_k, in_=bqk[ts(t, P)].rearrange("(p o) -> p o", o=1)
                )
                bq_t.append(bq_k)
            wv_t = []
            for k in range(KC):
                wv_k = wpool.tile([P, HPC * D], F16, name=f"wv{k}")
                nc.sync.dma_start(out=wv_k, in_=wv[ts(k, P), :])
                wv_t.append(wv_k)
            wpA = wpool.tile([P, EMBED], F16)
            nc.sync.dma_start(out=wpA, in_=wp[0:P, :])
            wpB = wpool.tile([P, EMBED], F16)
            nc.sync.dma_start(out=wpB, in_=wp[P : 2 * P, :])

            def qk_chunk(t, qkT_i, nq, bias_on_act=False):
                ps = psum.tile([P, 512], F32, tag="sc", name="ps_qk")
                for k in range(KC):
                    nc.tensor.matmul(
                        ps,
                        lhsT=wqk_t[k][:, ts(t, P)],
                        rhs=xT_t[k][:, ts(nq, 512)],
                        start=(k == 0),
                        stop=(k == KC - 1),
                    )
                if bias_on_act:
                    nc.scalar.activation(
                        out=qkT_i[:, ts(nq, 512)],
                        in_=ps,
                        func=mybir.ActivationFunctionType.Identity,
                        bias=bq_t[t],
                    )
                else:
                    nc.vector.tensor_scalar_add(
                        out=qkT_i[:, ts(nq, 512)], in0=ps, scalar1=bq_t[t]
                    )

            def qk_alloc(t):
                return qkpool.tile([P, S], F16, name=f"qkT{t}", tag=f"qkT{t}")

            # tile 0 [q0|q1] first, bias on the still-idle ScalarE; then
            # the combined [k0|k1] tile split into [k0|0],[0|k1] (DVE).
            qkT = {t: qk_alloc(t) for t in range(NT)}
            for t in (1, 2, 3, 4):
                half = slice(0, D) if t in (1,) else slice(D, P)
                zero = slice(D, P) if t in (1,) else slice(0, D)
                nc.vector.memset(qkT[t][zero, :], 0.0)
            for nq in range(NQ):
                qk_chunk(0, qkT[0], nq, bias_on_act=True)

            def qk_combined(tsrc, dst_a, dst_b, ba, bb, nq, shift_a=False):
                """Project w-tile tsrc = [a|b]; split psum halves into
                dst_a (a at 0:64 or 64:128) and dst_b (b at 64:128)."""
                ps = psum.tile([P, 512], F32, tag="sc", name="ps_qk")
                for k in range(KC):
                    nc.tensor.matmul(
                        ps,
                        lhsT=wqk_t[k][:, ts(tsrc, P)],
                        rhs=xT_t[k][:, ts(nq, 512)],
                        start=(k == 0),
                        stop=(k == KC - 1),
                    )
                a_rows = slice(D, P) if shift_a else slice(0, D)
                nc.vector.tensor_scalar_add(
                    out=dst_a[a_rows, ts(nq, 512)], in0=ps[0:D, :], scalar1=ba
                )
                nc.vector.tensor_scalar_add(
                    out=dst_b[D:P, ts(nq, 512)], in0=ps[D:P, :], scalar1=bb
                )

            def scores_step(kt, qt, pts):
                """Emit one Sk block of scoresT+exp; appends the pT tile."""
                sk = len(pts)
                pt = ptpool.tile([P, S], F16, name="pt", tag="pt")
                pts.append(pt)
                for g in range(2):
                    sps = psum.tile([P, 1024], F32, tag="sc", name="ps_s")
                    for j in range(2):
                        nc.tensor.matmul(
                            sps[:, ts(j, 512)],
                            lhsT=kt[:, ts(sk, P)],
                            rhs=qt[:, ds(g * 1024 + j * 512, 512)],
                            start=True,
                            stop=True,
                        )
                    nc.scalar.activation(
                        out=pt[:, ts(g, 1024)],
                        in_=sps,
                        func=mybir.ActivationFunctionType.Exp,
                        scale=float(D) ** -0.5,
                    )

            # v' tiles: [v(64) | ones(64)] per Sk block
            vp = []
            for h in range(HPC):
                vp_h = vpool.tile([P, NSK * P], F16, name=f"vp{h}", tag=f"vp{h}")
                nc.vector.memset(
                    vp_h.rearrange("p (s c) -> p s c", c=P)[:, :, D:P], 1.0
                )
                vp.append(vp_h)

            def v_chunk(st):
                vps = psum.tile([P, HPC * D], F32, tag="sc", name="ps_v")
                for k in range(KC):
                    nc.tensor.matmul(
                        vps,
                        lhsT=xT_t[k][:, ts(st, P)],
                        rhs=wv_t[k],
                        start=(k == 0),
                        stop=(k == KC - 1),
                    )
                for h in range(HPC):
                    nc.vector.tensor_copy(
                        out=vp[h][:, ds(st * P, D)], in_=vps[:, ts(h, D)]
                    )

            st01 = opool.tile([P, S], F16)   # heads 0 (rows 0:64) and 1 (64:128)
            outT2 = opool.tile([P, S], F16)  # head 2 (rows 0:64; 64:128 zeroed)
            nc.vector.memset(outT2[D:P, :], 0.0)

            def av_alloc():
                return [
                    psum.tile([P, 512], F32, tag="av", bufs=4, name="ps_av")
                    for _ in range(NQ)
                ]

            def av_step(h, pts, avs, sk):
                for nq in range(NQ):
                    nc.tensor.matmul(
                        avs[nq],
                        lhsT=vp[h][:, ts(sk, P)],
                        rhs=pts[sk][:, ts(nq, 512)],
                        start=(sk == 0),
                        stop=(sk == NSK - 1),
                    )

            def norm(h, avs):
                if h == 0:
                    dsts = [st01[0:D, ts(nq, 512)] for nq in range(NQ)]
                elif h == 1:
                    dsts = [st01[D:P, ts(nq, 512)] for nq in range(NQ)]
                else:
                    dsts = [outT2[0:D, ts(nq, 512)] for nq in range(NQ)]
                for nq in range(NQ):
                    # custom-DVE ops can't partition-shift: stage l at base 0
                    ll = rlpool.tile([D, 512], F32, name="ll", tag="ll")
                    nc.vector.tensor_copy(out=ll, in_=avs[nq][D:P, :])
                    rr = rlpool.tile([D, 512], F32, name="rr", tag="rr")
                    nc.vector.reciprocal_approx_fast(out=rr, in_=ll)
                    nc.vector.tensor_mul(
                        out=dsts[nq], in0=avs[nq][0:D, :], in1=rr
                    )

            # ---- head 0 scores, with the rest of qkv/v as interleaved
            # filler (same-tag PSUM slots are granted in emission order, so
            # fillers must be woven INTO the ACT-paced scores stream).
            fillers = []
            for nq in range(NQ):
                # w tile 1 = [k0|k1] -> qkT[1]=[k0|0], qkT[2]=[0|k1]
                fillers.append(
                    (qk_combined, (1, qkT[1], qkT[2], bq_t[1][0:D], bq_t[2][D:P], nq))
                )
                # w tile 2 = [q2|k2] -> qkT[3]=[0|q2] (shifted), qkT[4]=[0|k2]
                fillers.append(
                    (qk_combined, (2, qkT[3], qkT[4], bq_t[3][D:P], bq_t[4][D:P], nq, True))
                )
            for st in range(NSK):
                fillers.append((v_chunk, (st,)))
            pts_h = [[], [], []]
            fi = 0
            # prime: k0 chunk for sk block 0 must exist before the first
            # scores matmul
            f, a = fillers[fi]; f(*a); fi += 1
            for sk in range(NSK):
                scores_step(qkT[1], qkT[0], pts_h[0])
                take = 2 if sk < 12 else 1
                for _ in range(take):
                    if fi < len(fillers):
                        f, a = fillers[fi]
                        f(*a)
                        fi += 1
            while fi < len(fillers):
                f, a = fillers[fi]
                f(*a)
                fi += 1

            # ---- AV h interleaved with scores h+1 ----
            avs0 = av_alloc()
            for sk in range(NSK):
                av_step(0, pts_h[0], avs0, sk)
                scores_step(qkT[2], qkT[0], pts_h[1])
            norm(0, avs0)
            avs1 = av_alloc()
            for sk in range(NSK):
                av_step(1, pts_h[1], avs1, sk)
                scores_step(qkT[4], qkT[3], pts_h[2])
            norm(1, avs1)
            # head 2's AV borrows the "sc" slots (the scores stream is over,
            # so these grants drain right behind the last exps) - the "av"
            # slots are still held by head 1 until norm(1) completes.
            av2a = psum.tile([P, 1024], F32, tag="sc", name="av2a")
            av2b = psum.tile([P, 1024], F32, tag="sc", name="av2b")
            avs2 = [av2a[:, 0:512], av2a[:, 512:1024], av2b[:, 0:512], av2b[:, 512:1024]]
            for sk in range(NSK):
                av_step(2, pts_h[2], avs2, sk)
            norm(2, avs2)

            # ---- output projection: yT [768, S] = wp^T @ out_headsT ----
            for mt in range(EMBED // P):
                for nq in range(NQ):
                    yps = psum.tile([P, 512], F32, tag="sc", name="ps_y")
                    nc.tensor.matmul(
                        yps,
                        lhsT=wpA[:, ts(mt, P)],
                        rhs=st01[:, ts(nq, 512)],
                        start=True,
                        stop=False,
                    )
                    nc.tensor.matmul(
                        yps,
                        lhsT=wpB[:, ts(mt, P)],
                        rhs=outT2[:, ts(nq, 512)],
                        start=False,
                        stop=True,
                    )
                    ysb = rlpool.tile([P, 512], F32, name="ysb", tag="ysb", bufs=3)
                    nc.vector.tensor_copy(out=ysb, in_=yps)
                    nc.sync.dma_start(out=yT[ts(mt, P), ts(nq, 512)], in_=ysb)
    return nc


_CACHE = threading.Lock(), {}


def _get_nc():
    lock, cache = _CACHE
    with lock:
        if "nc" not in cache:
            nc = bacc.Bacc("TRN2", target_bir_lowering=False, debug=False)
            _build_kernel(nc)
            nc.compile()
            cache["nc"] = nc
        return cache["nc"]


def _shard_inputs(x, w_qkv, b_qkv, w_proj):
    """Build the 8 per-core input maps (host-side sharding/layout)."""
    in_maps = []
    for c in range(NCORES):
        b = c // 4
        hg = c % 4
        h0 = HPC * hg
        qc = [np.s_[D * (h0 + i) : D * (h0 + i + 1)] for i in range(HPC)]
        kc = [np.s_[EMBED + D * (h0 + i) : EMBED + D * (h0 + i + 1)] for i in range(HPC)]
        vc = [np.s_[2 * EMBED + D * (h0 + i) : 2 * EMBED + D * (h0 + i + 1)] for i in range(HPC)]

        # projected w tiles: [q0|q1], [k0|k1], [q2|k2]; bias vector is laid
        # out per DEVICE qkT tile t=0..4 = [q0|q1],[k0|0],[0|k1],[0|q2],[0|k2]
        w_qk = np.zeros((EMBED, 3 * P), dtype=np.float32)
        b_qk = np.zeros((NT * P,), dtype=np.float32)
        halves = [
            (0, 0, qc[0]), (0, 1, qc[1]),
            (1, 0, kc[0]), (1, 1, kc[1]),
            (2, 0, qc[2]), (2, 1, kc[2]),
        ]
        for t, half, cols in halves:
            w_qk[:, t * P + half * D : t * P + half * D + D] = w_qkv[:, cols]
        bias_halves = [
            (0, 0, qc[0]), (0, 1, qc[1]),
            (1, 0, kc[0]),
            (2, 1, kc[1]),
            (3, 1, qc[2]),
            (4, 1, kc[2]),
        ]
        for t, half, cols in bias_halves:
            b_qk[t * P + half * D : t * P + half * D + D] = b_qkv[cols]

        w_v = np.concatenate([w_qkv[:, s] for s in vc], axis=1)
        # w_proj rows for these heads; B half zero-padded to K=128
        w_p = np.zeros((2 * P, EMBED), dtype=np.float32)
        w_p[0:P] = w_proj[D * h0 : D * h0 + P]
        w_p[P : P + D] = w_proj[D * h0 + P : D * (h0 + HPC)]
        in_maps.append(
            {
                "xT": np.ascontiguousarray(x[b].T).astype(np.float16),
                "w_qk": w_qk.astype(np.float16),
                "b_qk": b_qk,
                "w_v": np.ascontiguousarray(w_v).astype(np.float16),
                "w_p": w_p.astype(np.float16),
            }
        )
    return in_maps


def kernel(x, w_qkv, b_qkv, w_proj, b_proj, _results_hook=None):
    x = np.asarray(x, dtype=np.float32)
    w_qkv = np.asarray(w_qkv, dtype=np.float32)
    b_qkv = np.asarray(b_qkv, dtype=np.float32)
    w_proj = np.asarray(w_proj, dtype=np.float32)
    b_proj = np.asarray(b_proj, dtype=np.float32)

    nc = _get_nc()
    in_maps = _shard_inputs(x, w_qkv, b_qkv, w_proj)
    res = run_bass_kernel_spmd(nc, in_maps, core_ids=list(range(NCORES)))
    if _results_hook is not None:
        _results_hook(res)

    # unshard: sum the 4 head-group partials per batch, add bias terms
    b_v = b_qkv[2 * EMBED :]
    bias_row = b_v @ w_proj + b_proj  # [768]
    out = np.empty((B, S, EMBED), dtype=np.float32)
    for b in range(B):
        acc = np.zeros((EMBED, S), dtype=np.float32)
        for hg in range(4):
            acc += res.results[4 * b + hg]["yT"]
        out[b] = acc.T + bias_row
    return out


# revision 16
# speedup vs baseline: 1.1200x; 1.1200x over previous
"""Multi-head attention (B=2, S=2048, E=768, H=12, D=64) on 8 NeuronCores.

Sharding: core c -> batch b = c//4, head group hg = c%4 (3 heads each).
Each core computes the qkv projection for its 3 heads, attention, and a
partial output projection (rows of w_proj for its heads). Host sums the 4
partials per batch and adds the bias terms (tensor-parallel unshard).

Device dataflow (everything transposed so no on-chip transposes are needed,
and every matmul has a full K=128 contraction — K=64 matmuls run the PE at
half clock):
  xT [768, 2048]  (host-pretransposed, fp16)
  qkT[t] = (w_qk_tile_t)^T @ xT -> [128, 2048] tiles t=0..4 with w columns
           laid out [q0|q1], [k0|0], [0|k1], [0|q2], [0|k2]: each head's
           scoresT matmul then uses a full-128-partition stationary whose
           zero half kills the other head's rows.
  v'   = xT_tile^T @ w_v -> per-head per-Sk-block [128, 128] blocks:
           cols 0:64 = v, cols 64:128 = 1.0, so the AV matmul also produces
           the softmax denominator replicated across partitions 64:128.
  pT   = exp(scoresT / 8)   (ScalarE, PSUM -> SBUF fp16)
  avT  = v'^T @ pT          -> [128, 512] PSUM per (head, Sq-chunk);
           rows 64:128 = denominator l
  outT = avT[0:64] * approx(1/l)   (DVE reciprocal_approx_fast, ~51 ULP)
  yT  += w_proj_part^T @ outT      -> [768, 2048] fp32 partial, DMA'd out.

Emission order keeps ScalarE (the 104us exp stream paces the middle of the
kernel) fed from ~15us on, and each head's AV matmuls are emitted after the
next head's scores so the Tile scheduler uses them to fill TensorE gaps.
"""

import threading

import numpy as np

import concourse.bass as bass
import concourse.tile as tile
from concourse import bacc, mybir
from concourse.bass import ts, ds
from concourse.bass_utils import run_bass_kernel_spmd

F32 = mybir.dt.float32
F16 = mybir.dt.float16

EMBED = 768
NH = 12
D = 64
B = 2
S = 2048
HPC = 3          # heads per core
NCORES = 8
P = 128
KC = EMBED // P  # 6 contraction chunks
NQ = S // 512    # 4 Sq chunks of 512
NSK = S // P     # 16 Sk blocks
NT = 5           # qk projection tiles


def _build_kernel(nc):
    xT = nc.dram_tensor("xT", [EMBED, S], F16, kind="ExternalInput").ap()
    wqk = nc.dram_tensor("w_qk", [EMBED, 3 * P], F16, kind="ExternalInput").ap()
    bqk = nc.dram_tensor("b_qk", [NT * P], F32, kind="ExternalInput").ap()
    wv = nc.dram_tensor("w_v", [EMBED, HPC * D], F16, kind="ExternalInput").ap()
    wp = nc.dram_tensor("w_p", [2 * P, EMBED], F16, kind="ExternalInput").ap()
    yT = nc.dram_tensor("yT", [EMBED, S], F32, kind="ExternalOutput").ap()

    with tile.TileContext(nc) as tc:
        with (
            tc.tile_pool(name="wpool", bufs=1) as wpool,
            tc.tile_pool(name="xpool", bufs=1) as xpool,
            tc.tile_pool(name="qkpool", bufs=1) as qkpool,
            tc.tile_pool(name="vpool", bufs=1) as vpool,
            tc.tile_pool(name="ptpool", bufs=26) as ptpool,
            tc.tile_pool(name="opool", bufs=1) as opool,
            tc.tile_pool(name="rlpool", bufs=4) as rlpool,
            tc.tile_pool(name="psum", bufs=3, space="PSUM") as psum,
        ):
            # ---- loads (xT interleaved with wqk so qk matmuls start early)
            wqk_t = []
            xT_t = []
            for k in range(KC):
                xT_k = xpool.tile([P, S], F16, name=f"xT{k}")
                nc.sync.dma_start(out=xT_k, in_=xT[ts(k, P), :])
                xT_t.append(xT_k)
                wqk_k = wpool.tile([P, 3 * P], F16, name=f"wqk{k}")
                nc.sync.dma_start(out=wqk_k, in_=wqk[ts(k, P), :])
                wqk_t.append(wqk_k)
            bq_t = []
            for t in range(NT):
                bq_k = wpool.tile([P, 1], F32, name=f"bq{t}")
                nc.sync.dma_start(
                    out=bq_k, in_=bqk[ts(t, P)].rearrange("(p o) -> p o", o=1)
                )
                bq_t.append(bq_k)
            wv_t = []
            for k in range(KC):
                wv_k = wpool.tile([P, HPC * D], F16, name=f"wv{k}")
                nc.sync.dma_start(out=wv_k, in_=wv[ts(k, P), :])
                wv_t.append(wv_k)
            wpA = wpool.tile([P, EMBED], F16)
            nc.sync.dma_start(out=wpA, in_=wp[0:P, :])
            wpB = wpool.tile([P, EMBED], F16)
            nc.sync.dma_start(out=wpB, in_=wp[P : 2 * P, :])

            def qk_chunk(t, qkT_i, nq, bias_on_act=False):
                ps = psum.tile([P, 512], F32, tag="sc", name="ps_qk")
                for k in range(KC):
                    nc.tensor.matmul(
                        ps,
                        lhsT=wqk_t[k][:, ts(t, P)],
                        rhs=xT_t[k][:, ts(nq, 512)],
                        start=(k == 0),
                        stop=(k == KC - 1),
                    )
                if bias_on_act:
                    nc.scalar.activation(
                        out=qkT_i[:, ts(nq, 512)],
                        in_=ps,
                        func=mybir.ActivationFunctionType.Identity,
                        bias=bq_t[t],
                    )
                else:
                    nc.vector.tensor_scalar_add(
                        out=qkT_i[:, ts(nq, 512)], in0=ps, scalar1=bq_t[t]
                    )

            def qk_alloc(t):
                return qkpool.tile([P, S], F16, name=f"qkT{t}", tag=f"qkT{t}")

            # tile 0 [q0|q1] first, bias on the still-idle ScalarE; then
            # the combined [k0|k1] tile split into [k0|0],[0|k1] (DVE).
            qkT = {t: qk_alloc(t) for t in range(NT)}
            for t in (1, 2, 3, 4):
                zero = slice(D, P) if t == 1 else slice(0, D)
                nc.gpsimd.memset(qkT[t][zero, :], 0.0)
            for nq in range(NQ):
                qk_chunk(0, qkT[0], nq, bias_on_act=True)

            def qk_combined(tsrc, dst_a, dst_b, ba, bb, nq, shift_a=False):
                """Project w-tile tsrc = [a|b]; split psum halves into
                dst_a (a at 0:64 or 64:128) and dst_b (b at 64:128)."""
                ps = psum.tile([P, 512], F32, tag="sc", name="ps_qk")
                for k in range(KC):
                    nc.tensor.matmul(
                        ps,
                        lhsT=wqk_t[k][:, ts(tsrc, P)],
                        rhs=xT_t[k][:, ts(nq, 512)],
                        start=(k == 0),
                        stop=(k == KC - 1),
                    )
                a_rows = slice(D, P) if shift_a else slice(0, D)
                nc.vector.tensor_scalar_add(
                    out=dst_a[a_rows, ts(nq, 512)], in0=ps[0:D, :], scalar1=ba
                )
                nc.vector.tensor_scalar_add(
                    out=dst_b[D:P, ts(nq, 512)], in0=ps[D:P, :], scalar1=bb
                )

            def scores_step(kt, qt, pts):
                """Emit one Sk block of scoresT+exp; appends the pT tile."""
                sk = len(pts)
                pt = ptpool.tile([P, S], F16, name="pt", tag="pt")
                pts.append(pt)
                for g in range(2):
                    sps = psum.tile([P, 1024], F32, tag="sc", name="ps_s")
                    for j in range(2):
                        nc.tensor.matmul(
                            sps[:, ts(j, 512)],
                            lhsT=kt[:, ts(sk, P)],
                            rhs=qt[:, ds(g * 1024 + j * 512, 512)],
                            start=True,
                            stop=True,
                        )
                    nc.scalar.activation(
                        out=pt[:, ts(g, 1024)],
                        in_=sps,
                        func=mybir.ActivationFunctionType.Exp,
                        scale=float(D) ** -0.5,
                    )

            # v' tiles: [v(64) | ones(64)] per Sk block
            vp = []
            for h in range(HPC):
                vp_h = vpool.tile([P, NSK * P], F16, name=f"vp{h}", tag=f"vp{h}")
                nc.gpsimd.memset(
                    vp_h.rearrange("p (s c) -> p s c", c=P)[:, :, D:P], 1.0
                )
                vp.append(vp_h)

            def v_chunk(st):
                vps = psum.tile([P, HPC * D], F32, tag="sc", name="ps_v")
                for k in range(KC):
                    nc.tensor.matmul(
                        vps,
                        lhsT=xT_t[k][:, ts(st, P)],
                        rhs=wv_t[k],
                        start=(k == 0),
                        stop=(k == KC - 1),
                    )
                for h in range(HPC):
                    nc.vector.tensor_copy(
                        out=vp[h][:, ds(st * P, D)], in_=vps[:, ts(h, D)]
                    )

            st01 = opool.tile([P, S], F16)   # heads 0 (rows 0:64) and 1 (64:128)
            outT2 = opool.tile([P, S], F16)  # head 2 (rows 0:64; 64:128 zeroed)
            nc.gpsimd.memset(outT2[D:P, :], 0.0)

            def norm_chunk(h, nq, av):
                if h == 0:
                    dst = st01[0:D, ts(nq, 512)]
                elif h == 1:
                    dst = st01[D:P, ts(nq, 512)]
                else:
                    dst = outT2[0:D, ts(nq, 512)]
                # custom-DVE ops can't partition-shift: stage l at base 0
                ll = rlpool.tile([D, 512], F32, name="ll", tag="ll")
                nc.vector.tensor_copy(out=ll, in_=av[D:P, :])
                rr = rlpool.tile([D, 512], F32, name="rr", tag="rr")
                nc.vector.reciprocal_approx_fast(out=rr, in_=ll)
                nc.vector.tensor_mul(out=dst, in0=av[0:D, :], in1=rr)

            def av_chunk(h, pts, nq):
                """One Sq chunk of AV^T for head h: accumulate over all Sk
                blocks into a single PSUM bank, then normalize the chunk."""
                av = psum.tile([P, 512], F32, tag="av", bufs=2, name="ps_av")
                for sk in range(NSK):
                    nc.tensor.matmul(
                        av,
                        lhsT=vp[h][:, ts(sk, P)],
                        rhs=pts[sk][:, ts(nq, 512)],
                        start=(sk == 0),
                        stop=(sk == NSK - 1),
                    )
                norm_chunk(h, nq, av)

            # ---- head 0 scores, with the rest of qkv/v as interleaved
            # filler (same-tag PSUM slots are granted in emission order, so
            # fillers must be woven INTO the ACT-paced scores stream).
            fillers = []
            for nq in range(NQ):
                # w tile 1 = [k0|k1] -> qkT[1]=[k0|0], qkT[2]=[0|k1]
                fillers.append(
                    (qk_combined, (1, qkT[1], qkT[2], bq_t[1][0:D], bq_t[2][D:P], nq))
                )
                # w tile 2 = [q2|k2] -> qkT[3]=[0|q2] (shifted), qkT[4]=[0|k2]
                fillers.append(
                    (qk_combined, (2, qkT[3], qkT[4], bq_t[3][D:P], bq_t[4][D:P], nq, True))
                )
            for st in range(NSK):
                fillers.append((v_chunk, (st,)))
            pts_h = [[], [], []]
            fi = 0
            # prime: k0 chunk for sk block 0 must exist before the first
            # scores matmul
            f, a = fillers[fi]; f(*a); fi += 1
            for sk in range(NSK):
                scores_step(qkT[1], qkT[0], pts_h[0])
                take = 2 if sk < 12 else 1
                for _ in range(take):
                    if fi < len(fillers):
                        f, a = fillers[fi]
                        f(*a)
                        fi += 1
            while fi < len(fillers):
                f, a = fillers[fi]
                f(*a)
                fi += 1

            # ---- AV h interleaved with scores h+1 ----
            # (an AV chunk's MMs are gated per-Sk-block on the exp stream,
            # so they pace themselves; one chunk is emitted per 4 score
            # steps to keep the "av" slot rotation aligned)
            for sk in range(NSK):
                scores_step(qkT[2], qkT[0], pts_h[1])
                if sk % 4 == 3:
                    av_chunk(0, pts_h[0], sk // 4)
            for sk in range(NSK):
                scores_step(qkT[4], qkT[3], pts_h[2])
                if sk % 4 == 3:
                    av_chunk(1, pts_h[1], sk // 4)
            for nq in range(NQ):
                av_chunk(2, pts_h[2], nq)

            # ---- output projection: yT [768, S] = wp^T @ out_headsT ----
            # nq-outer: chunk nq only needs outT2[:, nq] (head-2 norm chunk)
            for nq in range(NQ):
                for mt in range(EMBED // P):
                    yps = psum.tile([P, 512], F32, tag="sc", name="ps_y")
                    nc.tensor.matmul(
                        yps,
                        lhsT=wpA[:, ts(mt, P)],
                        rhs=st01[:, ts(nq, 512)],
                        start=True,
                        stop=False,
                    )
                    nc.tensor.matmul(
                        yps,
                        lhsT=wpB[:, ts(mt, P)],
                        rhs=outT2[:, ts(nq, 512)],
                        start=False,
                        stop=True,
                    )
                    ysb = rlpool.tile([P, 512], F32, name="ysb", tag="ysb", bufs=3)
                    nc.vector.tensor_copy(out=ysb, in_=yps)
                    nc.sync.dma_start(out=yT[ts(mt, P), ts(nq, 512)], in_=ysb)
    return nc


_CACHE = threading.Lock(), {}


def _get_nc():
    lock, cache = _CACHE
    with lock:
        if "nc" not in cache:
            nc = bacc.Bacc("TRN2", target_bir_lowering=False, debug=False)
            _build_kernel(nc)
            nc.compile()
            cache["nc"] = nc
        return cache["nc"]


def _shard_inputs(x, w_qkv, b_qkv, w_proj):
    """Build the 8 per-core input maps (host-side sharding/layout)."""
    in_maps = []
    for c in range(NCORES):
        b = c // 4
        hg = c % 4
        h0 = HPC * hg
        qc = [np.s_[D * (h0 + i) : D * (h0 + i + 1)] for i in range(HPC)]
        kc = [np.s_[EMBED + D * (h0 + i) : EMBED + D * (h0 + i + 1)] for i in range(HPC)]
        vc = [np.s_[2 * EMBED + D * (h0 + i) : 2 * EMBED + D * (h0 + i + 1)] for i in range(HPC)]

        # projected w tiles: [q0|q1], [k0|k1], [q2|k2]; bias vector is laid
        # out per DEVICE qkT tile t=0..4 = [q0|q1],[k0|0],[0|k1],[0|q2],[0|k2]
        w_qk = np.zeros((EMBED, 3 * P), dtype=np.float32)
        b_qk = np.zeros((NT * P,), dtype=np.float32)
        halves = [
            (0, 0, qc[0]), (0, 1, qc[1]),
            (1, 0, kc[0]), (1, 1, kc[1]),
            (2, 0, qc[2]), (2, 1, kc[2]),
        ]
        for t, half, cols in halves:
            w_qk[:, t * P + half * D : t * P + half * D + D] = w_qkv[:, cols]
        bias_halves = [
            (0, 0, qc[0]), (0, 1, qc[1]),
            (1, 0, kc[0]),
            (2, 1, kc[1]),
            (3, 1, qc[2]),
            (4, 1, kc[2]),
        ]
        for t, half, cols in bias_halves:
            b_qk[t * P + half * D : t * P + half * D + D] = b_qkv[cols]

        w_v = np.concatenate([w_qkv[:, s] for s in vc], axis=1)
        # w_proj rows for these heads; B half zero-padded to K=128
        w_p = np.zeros((2 * P, EMBED), dtype=np.float32)
        w_p[0:P] = w_proj[D * h0 : D * h0 + P]
        w_p[P : P + D] = w_proj[D * h0 + P : D * (h0 + HPC)]
        in_maps.append(
            {
                "xT": np.ascontiguousarray(x[b].T).astype(np.float16),
                "w_qk": w_qk.astype(np.float16),
                "b_qk": b_qk,
                "w_v": np.ascontiguousarray(w_v).astype(np.float16),
                "w_p": w_p.astype(np.float16),
            }
        )
    return in_maps


def kernel(x, w_qkv, b_qkv, w_proj, b_proj, _results_hook=None):
    x = np.asarray(x, dtype=np.float32)
    w_qkv = np.asarray(w_qkv, dtype=np.float32)
    b_qkv = np.asarray(b_qkv, dtype=np.float32)
    w_proj = np.asarray(w_proj, dtype=np.float32)
    b_proj = np.asarray(b_proj, dtype=np.float32)

    nc = _get_nc()
    in_maps = _shard_inputs(x, w_qkv, b_qkv, w_proj)
    res = run_bass_kernel_spmd(nc, in_maps, core_ids=list(range(NCORES)))
    if _results_hook is not None:
        _results_hook(res)

    # unshard: sum the 4 head-group partials per batch, add bias terms
    b_v = b_qkv[2 * EMBED :]
    bias_row = b_v @ w_proj + b_proj  # [768]
    out = np.empty((B, S, EMBED), dtype=np.float32)
    for b in range(B):
        acc = np.zeros((EMBED, S), dtype=np.float32)
        for hg in range(4):
            acc += res.results[4 * b + hg]["yT"]
        out[b] = acc.T + bias_row
    return out


# revision 17
# speedup vs baseline: 1.1675x; 1.0424x over previous
"""Multi-head attention (B=2, S=2048, E=768, H=12, D=64) on 8 NeuronCores.

Sharding: core c -> batch b = c//4, head group hg = c%4 (3 heads each).
Each core computes the qkv projection for its 3 heads, attention, and a
partial output projection (rows of w_proj for its heads). Host sums the 4
partials per batch and adds the bias terms (tensor-parallel unshard).

Device dataflow (everything transposed so no on-chip transposes are needed,
and every matmul has a full K=128 contraction — K=64 matmuls run the PE at
half clock):
  xT [768, 2048]  (host-pretransposed, fp16)
  qkT[t] = (w_qk_tile_t)^T @ xT -> [128, 2048] tiles t=0..4 with w columns
           laid out [q0|q1], [k0|0], [0|k1], [0|q2], [0|k2]: each head's
           scoresT matmul then uses a full-128-partition stationary whose
           zero half kills the other head's rows.
  v'   = xT_tile^T @ w_v -> per-head per-Sk-block [128, 128] blocks:
           cols 0:64 = v, cols 64:128 = 1.0, so the AV matmul also produces
           the softmax denominator replicated across partitions 64:128.
  pT   = exp(scoresT / 8)   (ScalarE, PSUM -> SBUF fp16)
  avT  = v'^T @ pT          -> [128, 512] PSUM per (head, Sq-chunk);
           rows 64:128 = denominator l
  outT = avT[0:64] * approx(1/l)   (DVE reciprocal_approx_fast, ~51 ULP)
  yT  += w_proj_part^T @ outT      -> [768, 2048] fp32 partial, DMA'd out.

Emission order keeps ScalarE (the 104us exp stream paces the middle of the
kernel) fed from ~15us on, and each head's AV matmuls are emitted after the
next head's scores so the Tile scheduler uses them to fill TensorE gaps.
"""

import threading

import numpy as np

import concourse.bass as bass
import concourse.tile as tile
from concourse import bacc, mybir
from concourse.bass import ts, ds
from concourse.bass_utils import run_bass_kernel_spmd

F32 = mybir.dt.float32
F16 = mybir.dt.float16

EMBED = 768
NH = 12
D = 64
B = 2
S = 2048
HPC = 3          # heads per core
NCORES = 8
P = 128
KC = EMBED // P  # 6 contraction chunks
NQ = S // 512    # 4 Sq chunks of 512
NSK = S // P     # 16 Sk blocks
NT = 5           # qk projection tiles


def _build_kernel(nc):
    xT = nc.dram_tensor("xT", [EMBED, S], F16, kind="ExternalInput").ap()
    wqk = nc.dram_tensor("w_qk", [EMBED, 3 * P], F16, kind="ExternalInput").ap()
    bqk = nc.dram_tensor("b_qk", [NT * P], F32, kind="ExternalInput").ap()
    wv = nc.dram_tensor("w_v", [EMBED, HPC * D], F16, kind="ExternalInput").ap()
    wp = nc.dram_tensor("w_p", [2 * P, EMBED], F16, kind="ExternalInput").ap()
    yT = nc.dram_tensor("yT", [EMBED, S], F32, kind="ExternalOutput").ap()

    with tile.TileContext(nc) as tc:
        with (
            tc.tile_pool(name="wpool", bufs=1) as wpool,
            tc.tile_pool(name="xpool", bufs=1) as xpool,
            tc.tile_pool(name="qkpool", bufs=1) as qkpool,
            tc.tile_pool(name="vpool", bufs=1) as vpool,
            tc.tile_pool(name="ptpool", bufs=26) as ptpool,
            tc.tile_pool(name="opool", bufs=1) as opool,
            tc.tile_pool(name="rlpool", bufs=4) as rlpool,
            tc.tile_pool(name="psum", bufs=3, space="PSUM") as psum,
        ):
            # ---- loads (xT interleaved with wqk so qk matmuls start early)
            wqk_t = []
            xT_t = []
            for k in range(KC):
                xT_k = xpool.tile([P, S], F16, name=f"xT{k}")
                nc.sync.dma_start(out=xT_k, in_=xT[ts(k, P), :])
                xT_t.append(xT_k)
                wqk_k = wpool.tile([P, 3 * P], F16, name=f"wqk{k}")
                nc.sync.dma_start(out=wqk_k, in_=wqk[ts(k, P), :])
                wqk_t.append(wqk_k)
            bq_t = []
            for t in range(NT):
                bq_k = wpool.tile([P, 1], F32, name=f"bq{t}")
                nc.sync.dma_start(
                    out=bq_k, in_=bqk[ts(t, P)].rearrange("(p o) -> p o", o=1)
                )
                bq_t.append(bq_k)
            wv_t = []
            for k in range(KC):
                wv_k = wpool.tile([P, HPC * D], F16, name=f"wv{k}")
                nc.sync.dma_start(out=wv_k, in_=wv[ts(k, P), :])
                wv_t.append(wv_k)
            wpA = wpool.tile([P, EMBED], F16)
            nc.sync.dma_start(out=wpA, in_=wp[0:P, :])
            wpB = wpool.tile([P, EMBED], F16)
            nc.sync.dma_start(out=wpB, in_=wp[P : 2 * P, :])

            def qk_chunk(t, qkT_i, nq, bias_on_act=False):
                ps = psum.tile([P, 512], F32, tag="sc", name="ps_qk")
                for k in range(KC):
                    nc.tensor.matmul(
                        ps,
                        lhsT=wqk_t[k][:, ts(t, P)],
                        rhs=xT_t[k][:, ts(nq, 512)],
                        start=(k == 0),
                        stop=(k == KC - 1),
                    )
                if bias_on_act:
                    nc.scalar.activation(
                        out=qkT_i[:, ts(nq, 512)],
                        in_=ps,
                        func=mybir.ActivationFunctionType.Identity,
                        bias=bq_t[t],
                    )
                else:
                    nc.vector.tensor_scalar_add(
                        out=qkT_i[:, ts(nq, 512)], in0=ps, scalar1=bq_t[t]
                    )

            def qk_alloc(t):
                return qkpool.tile([P, S], F16, name=f"qkT{t}", tag=f"qkT{t}")

            # tile 0 [q0|q1] first, bias on the still-idle ScalarE; then
            # the combined [k0|k1] tile split into [k0|0],[0|k1] (DVE).
            qkT = {t: qk_alloc(t) for t in range(NT)}
            for t in (1, 2, 3, 4):
                zero = slice(D, P) if t == 1 else slice(0, D)
                nc.gpsimd.memset(qkT[t][zero, :], 0.0)
            for nq in range(NQ):
                qk_chunk(0, qkT[0], nq, bias_on_act=True)

            def qk_combined(tsrc, dst_a, dst_b, ba, bb, nq, shift_a=False):
                """Project w-tile tsrc = [a|b]; split psum halves into
                dst_a (a at 0:64 or 64:128) and dst_b (b at 64:128)."""
                ps = psum.tile([P, 512], F32, tag="sc", name="ps_qk")
                for k in range(KC):
                    nc.tensor.matmul(
                        ps,
                        lhsT=wqk_t[k][:, ts(tsrc, P)],
                        rhs=xT_t[k][:, ts(nq, 512)],
                        start=(k == 0),
                        stop=(k == KC - 1),
                    )
                a_rows = slice(D, P) if shift_a else slice(0, D)
                nc.vector.tensor_scalar_add(
                    out=dst_a[a_rows, ts(nq, 512)], in0=ps[0:D, :], scalar1=ba
                )
                nc.vector.tensor_scalar_add(
                    out=dst_b[D:P, ts(nq, 512)], in0=ps[D:P, :], scalar1=bb
                )

            def scores_step(kt, qt, pts):
                """Emit one Sk block of scoresT+exp; appends the pT tile."""
                sk = len(pts)
                pt = ptpool.tile([P, S], F16, name="pt", tag="pt")
                pts.append(pt)
                for g in range(2):
                    sps = psum.tile([P, 1024], F32, tag="sc", name="ps_s")
                    for j in range(2):
                        nc.tensor.matmul(
                            sps[:, ts(j, 512)],
                            lhsT=kt[:, ts(sk, P)],
                            rhs=qt[:, ds(g * 1024 + j * 512, 512)],
                            start=True,
                            stop=True,
                        )
                    nc.scalar.activation(
                        out=pt[:, ts(g, 1024)],
                        in_=sps,
                        func=mybir.ActivationFunctionType.Exp,
                        scale=float(D) ** -0.5,
                    )

            # v' tiles: [v(64) | ones(64)] per Sk block
            vp = []
            for h in range(HPC):
                vp_h = vpool.tile([P, NSK * P], F16, name=f"vp{h}", tag=f"vp{h}")
                nc.gpsimd.memset(
                    vp_h.rearrange("p (s c) -> p s c", c=P)[:, :, D:P], 1.0
                )
                vp.append(vp_h)

            def v_chunk(st):
                vps = psum.tile([P, HPC * D], F32, tag="sc", name="ps_v")
                for k in range(KC):
                    nc.tensor.matmul(
                        vps,
                        lhsT=xT_t[k][:, ts(st, P)],
                        rhs=wv_t[k],
                        start=(k == 0),
                        stop=(k == KC - 1),
                    )
                for h in range(HPC):
                    nc.vector.tensor_copy(
                        out=vp[h][:, ds(st * P, D)], in_=vps[:, ts(h, D)]
                    )

            st01 = opool.tile([P, S], F16)   # heads 0 (rows 0:64) and 1 (64:128)
            outT2 = opool.tile([P, S], F16)  # head 2 (rows 0:64; 64:128 zeroed)
            nc.gpsimd.memset(outT2[D:P, :], 0.0)

            def norm_chunk(h, nq, av):
                if h == 0:
                    dst = st01[0:D, ts(nq, 512)]
                elif h == 1:
                    dst = st01[D:P, ts(nq, 512)]
                else:
                    dst = outT2[0:D, ts(nq, 512)]
                # custom-DVE ops can't partition-shift: stage l at base 0
                # (on ScalarE for the tail head, where ScalarE sits idle)
                ll = rlpool.tile([D, 512], F32, name="ll", tag="ll")
                if h == 2:
                    nc.scalar.copy(out=ll, in_=av[D:P, :])
                else:
                    nc.vector.tensor_copy(out=ll, in_=av[D:P, :])
                rr = rlpool.tile([D, 512], F32, name="rr", tag="rr")
                nc.vector.reciprocal_approx_fast(out=rr, in_=ll)
                nc.vector.tensor_mul(out=dst, in0=av[0:D, :], in1=rr)

            def av_chunk(h, pts, nq):
                """One Sq chunk of AV^T for head h: accumulate over all Sk
                blocks into a single PSUM bank, then normalize the chunk."""
                av = psum.tile([P, 512], F32, tag="av", bufs=2, name="ps_av")
                for sk in range(NSK):
                    nc.tensor.matmul(
                        av,
                        lhsT=vp[h][:, ts(sk, P)],
                        rhs=pts[sk][:, ts(nq, 512)],
                        start=(sk == 0),
                        stop=(sk == NSK - 1),
                    )
                norm_chunk(h, nq, av)

            # ---- head 0 scores, with the rest of qkv/v as interleaved
            # filler (same-tag PSUM slots are granted in emission order, so
            # fillers must be woven INTO the ACT-paced scores stream).
            fillers = []
            for nq in range(NQ):
                # w tile 1 = [k0|k1] -> qkT[1]=[k0|0], qkT[2]=[0|k1]
                fillers.append(
                    (qk_combined, (1, qkT[1], qkT[2], bq_t[1][0:D], bq_t[2][D:P], nq))
                )
                # w tile 2 = [q2|k2] -> qkT[3]=[0|q2] (shifted), qkT[4]=[0|k2]
                fillers.append(
                    (qk_combined, (2, qkT[3], qkT[4], bq_t[3][D:P], bq_t[4][D:P], nq, True))
                )
            for st in range(NSK):
                fillers.append((v_chunk, (st,)))
            pts_h = [[], [], []]
            fi = 0
            # prime: k0 chunk for sk block 0 must exist before the first
            # scores matmul
            f, a = fillers[fi]; f(*a); fi += 1
            for sk in range(NSK):
                scores_step(qkT[1], qkT[0], pts_h[0])
                take = 2 if sk < 12 else 1
                for _ in range(take):
                    if fi < len(fillers):
                        f, a = fillers[fi]
                        f(*a)
                        fi += 1
            while fi < len(fillers):
                f, a = fillers[fi]
                f(*a)
                fi += 1

            # ---- AV h interleaved with scores h+1 ----
            # (an AV chunk's MMs are gated per-Sk-block on the exp stream,
            # so they pace themselves; one chunk is emitted per 4 score
            # steps to keep the "av" slot rotation aligned)
            for sk in range(NSK):
                scores_step(qkT[2], qkT[0], pts_h[1])
                if sk % 4 == 3:
                    av_chunk(0, pts_h[0], sk // 4)
            for sk in range(NSK):
                scores_step(qkT[4], qkT[3], pts_h[2])
                if sk % 4 == 3:
                    av_chunk(1, pts_h[1], sk // 4)
            # ---- head-2 AV + output projection, interleaved per Sq chunk
            # (chunk nq of proj only needs outT2[:, nq]); PSUM->SBUF staging
            # of y on the now-idle ScalarE
            for nq in range(NQ):
                av_chunk(2, pts_h[2], nq)
                for mt in range(EMBED // P):
                    yps = psum.tile([P, 512], F32, tag="sc", name="ps_y")
                    nc.tensor.matmul(
                        yps,
                        lhsT=wpA[:, ts(mt, P)],
                        rhs=st01[:, ts(nq, 512)],
                        start=True,
                        stop=False,
                    )
                    nc.tensor.matmul(
                        yps,
                        lhsT=wpB[:, ts(mt, P)],
                        rhs=outT2[:, ts(nq, 512)],
                        start=False,
                        stop=True,
                    )
                    ysb = rlpool.tile([P, 512], F32, name="ysb", tag="ysb", bufs=3)
                    nc.scalar.copy(out=ysb, in_=yps)
                    nc.sync.dma_start(out=yT[ts(mt, P), ts(nq, 512)], in_=ysb)
    return nc


_CACHE = threading.Lock(), {}


def _get_nc():
    lock, cache = _CACHE
    with lock:
        if "nc" not in cache:
            nc = bacc.Bacc("TRN2", target_bir_lowering=False, debug=False)
            _build_kernel(nc)
            nc.compile()
            cache["nc"] = nc
        return cache["nc"]


def _shard_inputs(x, w_qkv, b_qkv, w_proj):
    """Build the 8 per-core input maps (host-side sharding/layout)."""
    in_maps = []
    for c in range(NCORES):
        b = c // 4
        hg = c % 4
        h0 = HPC * hg
        qc = [np.s_[D * (h0 + i) : D * (h0 + i + 1)] for i in range(HPC)]
        kc = [np.s_[EMBED + D * (h0 + i) : EMBED + D * (h0 + i + 1)] for i in range(HPC)]
        vc = [np.s_[2 * EMBED + D * (h0 + i) : 2 * EMBED + D * (h0 + i + 1)] for i in range(HPC)]

        # projected w tiles: [q0|q1], [k0|k1], [q2|k2]; bias vector is laid
        # out per DEVICE qkT tile t=0..4 = [q0|q1],[k0|0],[0|k1],[0|q2],[0|k2]
        w_qk = np.zeros((EMBED, 3 * P), dtype=np.float32)
        b_qk = np.zeros((NT * P,), dtype=np.float32)
        halves = [
            (0, 0, qc[0]), (0, 1, qc[1]),
            (1, 0, kc[0]), (1, 1, kc[1]),
            (2, 0, qc[2]), (2, 1, kc[2]),
        ]
        for t, half, cols in halves:
            w_qk[:, t * P + half * D : t * P + half * D + D] = w_qkv[:, cols]
        bias_halves = [
            (0, 0, qc[0]), (0, 1, qc[1]),
            (1, 0, kc[0]),
            (2, 1, kc[1]),
            (3, 1, qc[2]),
            (4, 1, kc[2]),
        ]
        for t, half, cols in bias_halves:
            b_qk[t * P + half * D : t * P + half * D + D] = b_qkv[cols]

        w_v = np.concatenate([w_qkv[:, s] for s in vc], axis=1)
        # w_proj rows for these heads; B half zero-padded to K=128
        w_p = np.zeros((2 * P, EMBED), dtype=np.float32)
        w_p[0:P] = w_proj[D * h0 : D * h0 + P]
        w_p[P : P + D] = w_proj[D * h0 + P : D * (h0 + HPC)]
        in_maps.append(
            {
                "xT": np.ascontiguousarray(x[b].T).astype(np.float16),
                "w_qk": w_qk.astype(np.float16),
                "b_qk": b_qk,
                "w_v": np.ascontiguousarray(w_v).astype(np.float16),
                "w_p": w_p.astype(np.float16),
            }
        )
    return in_maps


def kernel(x, w_qkv, b_qkv, w_proj, b_proj, _results_hook=None):
    x = np.asarray(x, dtype=np.float32)
    w_qkv = np.asarray(w_qkv, dtype=np.float32)
    b_qkv = np.asarray(b_qkv, dtype=np.float32)
    w_proj = np.asarray(w_proj, dtype=np.float32)
    b_proj = np.asarray(b_proj, dtype=np.float32)

    nc = _get_nc()
    in_maps = _shard_inputs(x, w_qkv, b_qkv, w_proj)
    res = run_bass_kernel_spmd(nc, in_maps, core_ids=list(range(NCORES)))
    if _results_hook is not None:
        _results_hook(res)

    # unshard: sum the 4 head-group partials per batch, add bias terms
    b_v = b_qkv[2 * EMBED :]
    bias_row = b_v @ w_proj + b_proj  # [768]
    out = np.empty((B, S, EMBED), dtype=np.float32)
    for b in range(B):
        acc = np.zeros((EMBED, S), dtype=np.float32)
        for hg in range(4):
            acc += res.results[4 * b + hg]["yT"]
        out[b] = acc.T + bias_row
    return out


# revision 19
# speedup vs baseline: 1.1851x; 1.0150x over previous
"""Multi-head attention (B=2, S=2048, E=768, H=12, D=64) on 8 NeuronCores.

Sharding: core c -> batch b = c//4, head group hg = c%4 (3 heads each).
Each core computes the qkv projection for its 3 heads, attention, and a
partial output projection (rows of w_proj for its heads). Host sums the 4
partials per batch and adds the bias terms (tensor-parallel unshard).

Device dataflow (everything transposed so no on-chip transposes are needed,
and every matmul has a full K=128 contraction — K=64 matmuls run the PE at
half clock):
  xT [768, 2048]  (host-pretransposed, fp16)
  qkT[t] = (w_qk_tile_t)^T @ xT -> [128, 2048] tiles t=0..4 with w columns
           laid out [q0|q1], [k0|0], [0|k1], [0|q2], [0|k2]: each head's
           scoresT matmul then uses a full-128-partition stationary whose
           zero half kills the other head's rows.
  v'   = xT_tile^T @ w_v -> per-head per-Sk-block [128, 128] blocks:
           cols 0:64 = v, cols 64:128 = 1.0, so the AV matmul also produces
           the softmax denominator replicated across partitions 64:128.
  pT   = exp(scoresT / 8)   (ScalarE, PSUM -> SBUF fp16)
  avT  = v'^T @ pT          -> [128, 512] PSUM per (head, Sq-chunk);
           rows 64:128 = denominator l
  outT = avT[0:64] * approx(1/l)   (DVE reciprocal_approx_fast, ~51 ULP)
  yT  += w_proj_part^T @ outT      -> [768, 2048] fp32 partial, DMA'd out.

Emission order keeps ScalarE (the 104us exp stream paces the middle of the
kernel) fed from ~15us on, and each head's AV matmuls are emitted after the
next head's scores so the Tile scheduler uses them to fill TensorE gaps.
"""

import threading

import numpy as np

import concourse.bass as bass
import concourse.tile as tile
from concourse import bacc, mybir
from concourse.bass import ts, ds
from concourse.bass_utils import run_bass_kernel_spmd

F32 = mybir.dt.float32
F16 = mybir.dt.float16

EMBED = 768
NH = 12
D = 64
B = 2
S = 2048
HPC = 3          # heads per core
NCORES = 8
P = 128
KC = EMBED // P  # 6 contraction chunks
NQ = S // 512    # 4 Sq chunks of 512
NSK = S // P     # 16 Sk blocks
NT = 5           # qk projection tiles


def _build_kernel(nc):
    xT = nc.dram_tensor("xT", [EMBED, S], F16, kind="ExternalInput").ap()
    wqk = nc.dram_tensor("w_qk", [EMBED, 3 * P], F16, kind="ExternalInput").ap()
    bqk = nc.dram_tensor("b_qk", [NT * P], F32, kind="ExternalInput").ap()
    wv = nc.dram_tensor("w_v", [EMBED, HPC * D], F16, kind="ExternalInput").ap()
    wp = nc.dram_tensor("w_p", [2 * P, EMBED], F16, kind="ExternalInput").ap()
    yT = nc.dram_tensor("yT", [EMBED, S], F32, kind="ExternalOutput").ap()

    with tile.TileContext(nc) as tc:
        with (
            tc.tile_pool(name="wpool", bufs=1) as wpool,
            tc.tile_pool(name="xpool", bufs=1) as xpool,
            tc.tile_pool(name="qkpool", bufs=1) as qkpool,
            tc.tile_pool(name="vpool", bufs=1) as vpool,
            tc.tile_pool(name="ptpool", bufs=26) as ptpool,
            tc.tile_pool(name="opool", bufs=1) as opool,
            tc.tile_pool(name="rlpool", bufs=4) as rlpool,
            tc.tile_pool(name="psum", bufs=3, space="PSUM") as psum,
        ):
            # ---- loads (xT interleaved with wqk so qk matmuls start early)
            wqk_t = []
            xT_t = []
            for k in range(KC):
                wqk_k = wpool.tile([P, 3 * P], F16, name=f"wqk{k}")
                nc.sync.dma_start(out=wqk_k, in_=wqk[ts(k, P), :])
                wqk_t.append(wqk_k)
                xT_k = xpool.tile([P, S], F16, name=f"xT{k}")
                nc.sync.dma_start(out=xT_k, in_=xT[ts(k, P), :])
                xT_t.append(xT_k)
                if k == 0:
                    # warm up the PE clock (HAM flips to 2.4 GHz after
                    # ~3.4us of sustained activity) while the DMAs stream
                    wps = psum.tile([P, 384], F32, tag="av", bufs=2, name="warm")
                    for r in range(16):
                        nc.tensor.matmul(
                            wps,
                            lhsT=wqk_t[0][:, 0:P],
                            rhs=wqk_t[0][:, 0:384],
                            start=(r == 0),
                            stop=(r == 15),
                        )
            bq_t = []
            for t in range(NT):
                bq_k = wpool.tile([P, 1], F32, name=f"bq{t}")
                nc.sync.dma_start(
                    out=bq_k, in_=bqk[ts(t, P)].rearrange("(p o) -> p o", o=1)
                )
                bq_t.append(bq_k)
            wexp = rlpool.tile([P, 1], F32, name="wexp", tag="wexp", bufs=1)
            nc.scalar.activation(
                out=wexp,
                in_=bq_t[0],
                func=mybir.ActivationFunctionType.Exp,
                scale=1.0,
            )
            wv_t = []
            for k in range(KC):
                wv_k = wpool.tile([P, HPC * D], F16, name=f"wv{k}")
                nc.sync.dma_start(out=wv_k, in_=wv[ts(k, P), :])
                wv_t.append(wv_k)
            wpA = wpool.tile([P, EMBED], F16)
            nc.sync.dma_start(out=wpA, in_=wp[0:P, :])
            wpB = wpool.tile([P, EMBED], F16)
            nc.sync.dma_start(out=wpB, in_=wp[P : 2 * P, :])

            def qk_chunk(t, qkT_i, nq, bias_on_act=False):
                ps = psum.tile([P, 512], F32, tag="sc", name="ps_qk")
                for k in range(KC):
                    nc.tensor.matmul(
                        ps,
                        lhsT=wqk_t[k][:, ts(t, P)],
                        rhs=xT_t[k][:, ts(nq, 512)],
                        start=(k == 0),
                        stop=(k == KC - 1),
                    )
                if bias_on_act:
                    nc.scalar.activation(
                        out=qkT_i[:, ts(nq, 512)],
                        in_=ps,
                        func=mybir.ActivationFunctionType.Identity,
                        bias=bq_t[t],
                    )
                else:
                    nc.vector.tensor_scalar_add(
                        out=qkT_i[:, ts(nq, 512)], in0=ps, scalar1=bq_t[t]
                    )

            def qk_alloc(t):
                return qkpool.tile([P, S], F16, name=f"qkT{t}", tag=f"qkT{t}")

            # tile 0 [q0|q1] first, bias on the still-idle ScalarE; then
            # the combined [k0|k1] tile split into [k0|0],[0|k1] (DVE).
            qkT = {t: qk_alloc(t) for t in range(NT)}
            for t in (1, 2, 3, 4):
                zero = slice(D, P) if t == 1 else slice(0, D)
                nc.gpsimd.memset(qkT[t][zero, :], 0.0)
            for nq in range(NQ):
                qk_chunk(0, qkT[0], nq, bias_on_act=True)

            def qk_combined(tsrc, dst_a, dst_b, ba, bb, nq, shift_a=False):
                """Project w-tile tsrc = [a|b]; split psum halves into
                dst_a (a at 0:64 or 64:128) and dst_b (b at 64:128)."""
                ps = psum.tile([P, 512], F32, tag="sc", name="ps_qk")
                for k in range(KC):
                    nc.tensor.matmul(
                        ps,
                        lhsT=wqk_t[k][:, ts(tsrc, P)],
                        rhs=xT_t[k][:, ts(nq, 512)],
                        start=(k == 0),
                        stop=(k == KC - 1),
                    )
                a_rows = slice(D, P) if shift_a else slice(0, D)
                nc.vector.tensor_scalar_add(
                    out=dst_a[a_rows, ts(nq, 512)], in0=ps[0:D, :], scalar1=ba
                )
                nc.vector.tensor_scalar_add(
                    out=dst_b[D:P, ts(nq, 512)], in0=ps[D:P, :], scalar1=bb
                )

            def scores_step(kt, qt, pts):
                """Emit one Sk block of scoresT+exp; appends the pT tile."""
                sk = len(pts)
                pt = ptpool.tile([P, S], F16, name="pt", tag="pt")
                pts.append(pt)
                for g in range(2):
                    sps = psum.tile([P, 1024], F32, tag="sc", name="ps_s")
                    for j in range(2):
                        nc.tensor.matmul(
                            sps[:, ts(j, 512)],
                            lhsT=kt[:, ts(sk, P)],
                            rhs=qt[:, ds(g * 1024 + j * 512, 512)],
                            start=True,
                            stop=True,
                        )
                    nc.scalar.activation(
                        out=pt[:, ts(g, 1024)],
                        in_=sps,
                        func=mybir.ActivationFunctionType.Exp,
                        scale=float(D) ** -0.5,
                    )

            # v' tiles: [v(64) | ones(64)] per Sk block
            vp = []
            for h in range(HPC):
                vp_h = vpool.tile([P, NSK * P], F16, name=f"vp{h}", tag=f"vp{h}")
                nc.gpsimd.memset(
                    vp_h.rearrange("p (s c) -> p s c", c=P)[:, :, D:P], 1.0
                )
                vp.append(vp_h)

            def v_chunk(st):
                vps = psum.tile([P, HPC * D], F32, tag="sc", name="ps_v")
                for k in range(KC):
                    nc.tensor.matmul(
                        vps,
                        lhsT=xT_t[k][:, ts(st, P)],
                        rhs=wv_t[k],
                        start=(k == 0),
                        stop=(k == KC - 1),
                    )
                for h in range(HPC):
                    nc.vector.tensor_copy(
                        out=vp[h][:, ds(st * P, D)], in_=vps[:, ts(h, D)]
                    )

            st01 = opool.tile([P, S], F16)   # heads 0 (rows 0:64) and 1 (64:128)
            outT2 = opool.tile([P, S], F16)  # head 2 (rows 0:64; 64:128 zeroed)
            nc.gpsimd.memset(outT2[D:P, :], 0.0)

            def norm_chunk(h, nq, av):
                if h == 0:
                    dst = st01[0:D, ts(nq, 512)]
                elif h == 1:
                    dst = st01[D:P, ts(nq, 512)]
                else:
                    dst = outT2[0:D, ts(nq, 512)]
                # custom-DVE ops can't partition-shift: stage l at base 0
                # (on ScalarE for the tail head, where ScalarE sits idle)
                ll = rlpool.tile([D, 512], F32, name="ll", tag="ll")
                if h == 2:
                    nc.scalar.copy(out=ll, in_=av[D:P, :])
                else:
                    nc.vector.tensor_copy(out=ll, in_=av[D:P, :])
                rr = rlpool.tile([D, 512], F32, name="rr", tag="rr")
                nc.vector.reciprocal_approx_fast(out=rr, in_=ll)
                nc.vector.tensor_mul(out=dst, in0=av[0:D, :], in1=rr)

            def av_chunk(h, pts, nq):
                """One Sq chunk of AV^T for head h: accumulate over all Sk
                blocks into a single PSUM bank, then normalize the chunk."""
                av = psum.tile([P, 512], F32, tag="av", bufs=2, name="ps_av")
                for sk in range(NSK):
                    nc.tensor.matmul(
                        av,
                        lhsT=vp[h][:, ts(sk, P)],
                        rhs=pts[sk][:, ts(nq, 512)],
                        start=(sk == 0),
                        stop=(sk == NSK - 1),
                    )
                norm_chunk(h, nq, av)

            # ---- head 0 scores, with the rest of qkv/v as interleaved
            # filler (same-tag PSUM slots are granted in emission order, so
            # fillers must be woven INTO the ACT-paced scores stream).
            fillers = []
            for nq in range(NQ):
                # w tile 1 = [k0|k1] -> qkT[1]=[k0|0], qkT[2]=[0|k1]
                fillers.append(
                    (qk_combined, (1, qkT[1], qkT[2], bq_t[1][0:D], bq_t[2][D:P], nq))
                )
                # w tile 2 = [q2|k2] -> qkT[3]=[0|q2] (shifted), qkT[4]=[0|k2]
                fillers.append(
                    (qk_combined, (2, qkT[3], qkT[4], bq_t[3][D:P], bq_t[4][D:P], nq, True))
                )
            for st in range(NSK):
                fillers.append((v_chunk, (st,)))
            pts_h = [[], [], []]
            fi = 0
            # prime: k0 chunk for sk block 0 must exist before the first
            # scores matmul
            f, a = fillers[fi]; f(*a); fi += 1
            for sk in range(NSK):
                scores_step(qkT[1], qkT[0], pts_h[0])
                take = 2 if sk < 12 else 1
                for _ in range(take):
                    if fi < len(fillers):
                        f, a = fillers[fi]
                        f(*a)
                        fi += 1
            while fi < len(fillers):
                f, a = fillers[fi]
                f(*a)
                fi += 1

            # ---- AV h interleaved with scores h+1 ----
            # (an AV chunk's MMs are gated per-Sk-block on the exp stream,
            # so they pace themselves; one chunk is emitted per 4 score
            # steps to keep the "av" slot rotation aligned)
            for sk in range(NSK):
                scores_step(qkT[2], qkT[0], pts_h[1])
                if sk % 4 == 3:
                    av_chunk(0, pts_h[0], sk // 4)
            for sk in range(NSK):
                scores_step(qkT[4], qkT[3], pts_h[2])
                if sk % 4 == 3:
                    av_chunk(1, pts_h[1], sk // 4)
            # ---- head-2 AV + output projection, interleaved per Sq chunk
            # (chunk nq of proj only needs outT2[:, nq]); PSUM->SBUF staging
            # of y on the now-idle ScalarE
            for nq in range(NQ):
                av_chunk(2, pts_h[2], nq)
                for mt in range(EMBED // P):
                    yps = psum.tile([P, 512], F32, tag="sc", name="ps_y")
                    nc.tensor.matmul(
                        yps,
                        lhsT=wpA[:, ts(mt, P)],
                        rhs=st01[:, ts(nq, 512)],
                        start=True,
                        stop=False,
                    )
                    nc.tensor.matmul(
                        yps,
                        lhsT=wpB[:, ts(mt, P)],
                        rhs=outT2[:, ts(nq, 512)],
                        start=False,
                        stop=True,
                    )
                    ysb = rlpool.tile([P, 512], F32, name="ysb", tag="ysb", bufs=3)
                    nc.scalar.copy(out=ysb, in_=yps)
                    nc.sync.dma_start(out=yT[ts(mt, P), ts(nq, 512)], in_=ysb)
    return nc


_CACHE = threading.Lock(), {}


def _get_nc():
    lock, cache = _CACHE
    with lock:
        if "nc" not in cache:
            nc = bacc.Bacc("TRN2", target_bir_lowering=False, debug=False)
            _build_kernel(nc)
            nc.compile()
            cache["nc"] = nc
        return cache["nc"]


def _shard_inputs(x, w_qkv, b_qkv, w_proj):
    """Build the 8 per-core input maps (host-side sharding/layout)."""
    in_maps = []
    for c in range(NCORES):
        b = c // 4
        hg = c % 4
        h0 = HPC * hg
        qc = [np.s_[D * (h0 + i) : D * (h0 + i + 1)] for i in range(HPC)]
        kc = [np.s_[EMBED + D * (h0 + i) : EMBED + D * (h0 + i + 1)] for i in range(HPC)]
        vc = [np.s_[2 * EMBED + D * (h0 + i) : 2 * EMBED + D * (h0 + i + 1)] for i in range(HPC)]

        # projected w tiles: [q0|q1], [k0|k1], [q2|k2]; bias vector is laid
        # out per DEVICE qkT tile t=0..4 = [q0|q1],[k0|0],[0|k1],[0|q2],[0|k2]
        w_qk = np.zeros((EMBED, 3 * P), dtype=np.float32)
        b_qk = np.zeros((NT * P,), dtype=np.float32)
        halves = [
            (0, 0, qc[0]), (0, 1, qc[1]),
            (1, 0, kc[0]), (1, 1, kc[1]),
            (2, 0, qc[2]), (2, 1, kc[2]),
        ]
        for t, half, cols in halves:
            w_qk[:, t * P + half * D : t * P + half * D + D] = w_qkv[:, cols]
        bias_halves = [
            (0, 0, qc[0]), (0, 1, qc[1]),
            (1, 0, kc[0]),
            (2, 1, kc[1]),
            (3, 1, qc[2]),
            (4, 1, kc[2]),
        ]
        for t, half, cols in bias_halves:
            b_qk[t * P + half * D : t * P + half * D + D] = b_qkv[cols]

        w_v = np.concatenate([w_qkv[:, s] for s in vc], axis=1)
        # w_proj rows for these heads; B half zero-padded to K=128
        w_p = np.zeros((2 * P, EMBED), dtype=np.float32)
        w_p[0:P] = w_proj[D * h0 : D * h0 + P]
        w_p[P : P + D] = w_proj[D * h0 + P : D * (h0 + HPC)]
        in_maps.append(
            {
                "xT": np.ascontiguousarray(x[b].T).astype(np.float16),
                "w_qk": w_qk.astype(np.float16),
                "b_qk": b_qk,
                "w_v": np.ascontiguousarray(w_v).astype(np.float16),
                "w_p": w_p.astype(np.float16),
            }
        )
    return in_maps


def kernel(x, w_qkv, b_qkv, w_proj, b_proj, _results_hook=None):
    x = np.asarray(x, dtype=np.float32)
    w_qkv = np.asarray(w_qkv, dtype=np.float32)
    b_qkv = np.asarray(b_qkv, dtype=np.float32)
    w_proj = np.asarray(w_proj, dtype=np.float32)
    b_proj = np.asarray(b_proj, dtype=np.float32)

    nc = _get_nc()
    in_maps = _shard_inputs(x, w_qkv, b_qkv, w_proj)
    res = run_bass_kernel_spmd(nc, in_maps, core_ids=list(range(NCORES)))
    if _results_hook is not None:
        _results_hook(res)

    # unshard: sum the 4 head-group partials per batch, add bias terms
    b_v = b_qkv[2 * EMBED :]
    bias_row = b_v @ w_proj + b_proj  # [768]
    out = np.empty((B, S, EMBED), dtype=np.float32)
    for b in range(B):
        acc = np.zeros((EMBED, S), dtype=np.float32)
        for hg in range(4):
            acc += res.results[4 * b + hg]["yT"]
        out[b] = acc.T + bias_row
    return out


# revision 20
# speedup vs baseline: 1.2101x; 1.0212x over previous
"""Multi-head attention (B=2, S=2048, E=768, H=12, D=64) on 8 NeuronCores.

Sharding: core c -> batch b = c//4, head group hg = c%4 (3 heads each).
Each core computes the qkv projection for its 3 heads, attention, and a
partial output projection (rows of w_proj for its heads). Host sums the 4
partials per batch and adds the bias terms (tensor-parallel unshard).

Device dataflow (everything transposed so no on-chip transposes are needed,
and every matmul has a full K=128 contraction — K=64 matmuls run the PE at
half clock):
  xT [768, 2048]  (host-pretransposed, fp16)
  qkT[t] = (w_qk_tile_t)^T @ xT -> [128, 2048] tiles t=0..4 with w columns
           laid out [q0|q1], [k0|0], [0|k1], [0|q2], [0|k2]: each head's
           scoresT matmul then uses a full-128-partition stationary whose
           zero half kills the other head's rows.
  v'   = xT_tile^T @ w_v -> per-head per-Sk-block [128, 128] blocks:
           cols 0:64 = v, cols 64:128 = 1.0, so the AV matmul also produces
           the softmax denominator replicated across partitions 64:128.
  pT   = exp(scoresT / 8)   (ScalarE, PSUM -> SBUF fp16)
  avT  = v'^T @ pT          -> [128, 512] PSUM per (head, Sq-chunk);
           rows 64:128 = denominator l
  outT = avT[0:64] * approx(1/l)   (DVE reciprocal_approx_fast, ~51 ULP)
  yT  += w_proj_part^T @ outT      -> [768, 2048] fp32 partial, DMA'd out.

Emission order keeps ScalarE (the 104us exp stream paces the middle of the
kernel) fed from ~15us on, and each head's AV matmuls are emitted after the
next head's scores so the Tile scheduler uses them to fill TensorE gaps.
"""

import threading

import numpy as np

import concourse.bass as bass
import concourse.tile as tile
from concourse import bacc, mybir
from concourse.bass import ts, ds
from concourse.bass_utils import run_bass_kernel_spmd

F32 = mybir.dt.float32
F16 = mybir.dt.float16

EMBED = 768
NH = 12
D = 64
B = 2
S = 2048
HPC = 3          # heads per core
NCORES = 8
P = 128
KC = EMBED // P  # 6 contraction chunks
NQ = S // 512    # 4 Sq chunks of 512
NSK = S // P     # 16 Sk blocks
NT = 5           # qk projection tiles


def _build_kernel(nc):
    xT = nc.dram_tensor("xT", [EMBED, S], F16, kind="ExternalInput").ap()
    wqk = nc.dram_tensor("w_qk", [EMBED, 3 * P], F16, kind="ExternalInput").ap()
    bqk = nc.dram_tensor("b_qk", [NT * P], F32, kind="ExternalInput").ap()
    wv = nc.dram_tensor("w_v", [EMBED, HPC * D], F16, kind="ExternalInput").ap()
    wp = nc.dram_tensor("w_p", [2 * P, EMBED], F16, kind="ExternalInput").ap()
    yT = nc.dram_tensor("yT", [EMBED, S], F16, kind="ExternalOutput").ap()

    with tile.TileContext(nc) as tc:
        with (
            tc.tile_pool(name="wpool", bufs=1) as wpool,
            tc.tile_pool(name="xpool", bufs=1) as xpool,
            tc.tile_pool(name="qkpool", bufs=1) as qkpool,
            tc.tile_pool(name="vpool", bufs=1) as vpool,
            tc.tile_pool(name="ptpool", bufs=26) as ptpool,
            tc.tile_pool(name="opool", bufs=1) as opool,
            tc.tile_pool(name="rlpool", bufs=4) as rlpool,
            tc.tile_pool(name="psum", bufs=3, space="PSUM") as psum,
        ):
            # ---- loads (xT interleaved with wqk so qk matmuls start early)
            wqk_t = []
            xT_t = []
            for k in range(KC):
                wqk_k = wpool.tile([P, 3 * P], F16, name=f"wqk{k}")
                nc.sync.dma_start(out=wqk_k, in_=wqk[ts(k, P), :])
                wqk_t.append(wqk_k)
                xT_k = xpool.tile([P, S], F16, name=f"xT{k}")
                nc.sync.dma_start(out=xT_k, in_=xT[ts(k, P), :])
                xT_t.append(xT_k)
                if k == 0:
                    # warm up the PE clock (HAM flips to 2.4 GHz after
                    # ~3.4us of sustained activity) while the DMAs stream
                    wps = psum.tile([P, 384], F32, tag="av", bufs=2, name="warm")
                    for r in range(16):
                        nc.tensor.matmul(
                            wps,
                            lhsT=wqk_t[0][:, 0:P],
                            rhs=wqk_t[0][:, 0:384],
                            start=(r == 0),
                            stop=(r == 15),
                        )
            bq_t = []
            for t in range(NT):
                bq_k = wpool.tile([P, 1], F32, name=f"bq{t}")
                nc.sync.dma_start(
                    out=bq_k, in_=bqk[ts(t, P)].rearrange("(p o) -> p o", o=1)
                )
                bq_t.append(bq_k)
            wexp = rlpool.tile([P, 1], F32, name="wexp", tag="wexp", bufs=1)
            nc.scalar.activation(
                out=wexp,
                in_=bq_t[0],
                func=mybir.ActivationFunctionType.Exp,
                scale=1.0,
            )
            wv_t = []
            for k in range(KC):
                wv_k = wpool.tile([P, HPC * D], F16, name=f"wv{k}")
                nc.sync.dma_start(out=wv_k, in_=wv[ts(k, P), :])
                wv_t.append(wv_k)
            wpA = wpool.tile([P, EMBED], F16)
            nc.sync.dma_start(out=wpA, in_=wp[0:P, :])
            wpB = wpool.tile([P, EMBED], F16)
            nc.sync.dma_start(out=wpB, in_=wp[P : 2 * P, :])

            def qk_chunk(t, qkT_i, nq, bias_on_act=False):
                ps = psum.tile([P, 512], F32, tag="sc", name="ps_qk")
                for k in range(KC):
                    nc.tensor.matmul(
                        ps,
                        lhsT=wqk_t[k][:, ts(t, P)],
                        rhs=xT_t[k][:, ts(nq, 512)],
                        start=(k == 0),
                        stop=(k == KC - 1),
                    )
                if bias_on_act:
                    nc.scalar.activation(
                        out=qkT_i[:, ts(nq, 512)],
                        in_=ps,
                        func=mybir.ActivationFunctionType.Identity,
                        bias=bq_t[t],
                    )
                else:
                    nc.vector.tensor_scalar_add(
                        out=qkT_i[:, ts(nq, 512)], in0=ps, scalar1=bq_t[t]
                    )

            def qk_alloc(t):
                return qkpool.tile([P, S], F16, name=f"qkT{t}", tag=f"qkT{t}")

            # tile 0 [q0|q1] first, bias on the still-idle ScalarE; then
            # the combined [k0|k1] tile split into [k0|0],[0|k1] (DVE).
            qkT = {t: qk_alloc(t) for t in range(NT)}
            for t in (1, 2, 3, 4):
                zero = slice(D, P) if t == 1 else slice(0, D)
                nc.gpsimd.memset(qkT[t][zero, :], 0.0)
            for nq in range(NQ):
                qk_chunk(0, qkT[0], nq, bias_on_act=True)

            def qk_combined(tsrc, dst_a, dst_b, ba, bb, nq, shift_a=False):
                """Project w-tile tsrc = [a|b]; split psum halves into
                dst_a (a at 0:64 or 64:128) and dst_b (b at 64:128)."""
                ps = psum.tile([P, 512], F32, tag="sc", name="ps_qk")
                for k in range(KC):
                    nc.tensor.matmul(
                        ps,
                        lhsT=wqk_t[k][:, ts(tsrc, P)],
                        rhs=xT_t[k][:, ts(nq, 512)],
                        start=(k == 0),
                        stop=(k == KC - 1),
                    )
                a_rows = slice(D, P) if shift_a else slice(0, D)
                nc.vector.tensor_scalar_add(
                    out=dst_a[a_rows, ts(nq, 512)], in0=ps[0:D, :], scalar1=ba
                )
                nc.vector.tensor_scalar_add(
                    out=dst_b[D:P, ts(nq, 512)], in0=ps[D:P, :], scalar1=bb
                )

            def scores_step(kt, qt, pts):
                """Emit one Sk block of scoresT+exp; appends the pT tile."""
                sk = len(pts)
                pt = ptpool.tile([P, S], F16, name="pt", tag="pt")
                pts.append(pt)
                for g in range(2):
                    sps = psum.tile([P, 1024], F32, tag="sc", name="ps_s")
                    for j in range(2):
                        nc.tensor.matmul(
                            sps[:, ts(j, 512)],
                            lhsT=kt[:, ts(sk, P)],
                            rhs=qt[:, ds(g * 1024 + j * 512, 512)],
                            start=True,
                            stop=True,
                        )
                    nc.scalar.activation(
                        out=pt[:, ts(g, 1024)],
                        in_=sps,
                        func=mybir.ActivationFunctionType.Exp,
                        scale=float(D) ** -0.5,
                    )

            # v' tiles: [v(64) | ones(64)] per Sk block
            vp = []
            for h in range(HPC):
                vp_h = vpool.tile([P, NSK * P], F16, name=f"vp{h}", tag=f"vp{h}")
                nc.gpsimd.memset(
                    vp_h.rearrange("p (s c) -> p s c", c=P)[:, :, D:P], 1.0
                )
                vp.append(vp_h)

            def v_chunk(st):
                vps = psum.tile([P, HPC * D], F32, tag="sc", name="ps_v")
                for k in range(KC):
                    nc.tensor.matmul(
                        vps,
                        lhsT=xT_t[k][:, ts(st, P)],
                        rhs=wv_t[k],
                        start=(k == 0),
                        stop=(k == KC - 1),
                    )
                for h in range(HPC):
                    nc.vector.tensor_copy(
                        out=vp[h][:, ds(st * P, D)], in_=vps[:, ts(h, D)]
                    )

            st01 = opool.tile([P, S], F16)   # heads 0 (rows 0:64) and 1 (64:128)
            outT2 = opool.tile([P, S], F16)  # head 2 (rows 0:64; 64:128 zeroed)
            nc.gpsimd.memset(outT2[D:P, :], 0.0)

            def norm_chunk(h, nq, av):
                if h == 0:
                    dst = st01[0:D, ts(nq, 512)]
                elif h == 1:
                    dst = st01[D:P, ts(nq, 512)]
                else:
                    dst = outT2[0:D, ts(nq, 512)]
                # custom-DVE ops can't partition-shift: stage l at base 0
                # (on ScalarE for the tail head, where ScalarE sits idle)
                ll = rlpool.tile([D, 512], F32, name="ll", tag="ll")
                if h == 2:
                    nc.scalar.copy(out=ll, in_=av[D:P, :])
                else:
                    nc.vector.tensor_copy(out=ll, in_=av[D:P, :])
                rr = rlpool.tile([D, 512], F32, name="rr", tag="rr")
                nc.vector.reciprocal_approx_fast(out=rr, in_=ll)
                nc.vector.tensor_mul(out=dst, in0=av[0:D, :], in1=rr)

            def av_chunk(h, pts, nq):
                """One Sq chunk of AV^T for head h: accumulate over all Sk
                blocks into a single PSUM bank, then normalize the chunk."""
                av = psum.tile([P, 512], F32, tag="av", bufs=2, name="ps_av")
                for sk in range(NSK):
                    nc.tensor.matmul(
                        av,
                        lhsT=vp[h][:, ts(sk, P)],
                        rhs=pts[sk][:, ts(nq, 512)],
                        start=(sk == 0),
                        stop=(sk == NSK - 1),
                    )
                norm_chunk(h, nq, av)

            # ---- head 0 scores, with the rest of qkv/v as interleaved
            # filler (same-tag PSUM slots are granted in emission order, so
            # fillers must be woven INTO the ACT-paced scores stream).
            fillers = []
            for nq in range(NQ):
                # w tile 1 = [k0|k1] -> qkT[1]=[k0|0], qkT[2]=[0|k1]
                fillers.append(
                    (qk_combined, (1, qkT[1], qkT[2], bq_t[1][0:D], bq_t[2][D:P], nq))
                )
                # w tile 2 = [q2|k2] -> qkT[3]=[0|q2] (shifted), qkT[4]=[0|k2]
                fillers.append(
                    (qk_combined, (2, qkT[3], qkT[4], bq_t[3][D:P], bq_t[4][D:P], nq, True))
                )
            for st in range(NSK):
                fillers.append((v_chunk, (st,)))
            pts_h = [[], [], []]
            fi = 0
            # prime: k0 chunk for sk block 0 must exist before the first
            # scores matmul
            f, a = fillers[fi]; f(*a); fi += 1
            for sk in range(NSK):
                scores_step(qkT[1], qkT[0], pts_h[0])
                take = 2 if sk < 12 else 1
                for _ in range(take):
                    if fi < len(fillers):
                        f, a = fillers[fi]
                        f(*a)
                        fi += 1
            while fi < len(fillers):
                f, a = fillers[fi]
                f(*a)
                fi += 1

            # ---- AV h interleaved with scores h+1 ----
            # (an AV chunk's MMs are gated per-Sk-block on the exp stream,
            # so they pace themselves; one chunk is emitted per 4 score
            # steps to keep the "av" slot rotation aligned)
            for sk in range(NSK):
                scores_step(qkT[2], qkT[0], pts_h[1])
                if sk % 4 == 3:
                    av_chunk(0, pts_h[0], sk // 4)
            for sk in range(NSK):
                scores_step(qkT[4], qkT[3], pts_h[2])
                if sk % 4 == 3:
                    av_chunk(1, pts_h[1], sk // 4)
            # ---- head-2 AV + output projection, interleaved per Sq chunk
            # (chunk nq of proj only needs outT2[:, nq]); PSUM->SBUF staging
            # of y on the now-idle ScalarE
            for nq in range(NQ):
                av_chunk(2, pts_h[2], nq)
                for mt in range(EMBED // P):
                    yps = psum.tile([P, 512], F32, tag="sc", name="ps_y")
                    nc.tensor.matmul(
                        yps,
                        lhsT=wpA[:, ts(mt, P)],
                        rhs=st01[:, ts(nq, 512)],
                        start=True,
                        stop=False,
                    )
                    nc.tensor.matmul(
                        yps,
                        lhsT=wpB[:, ts(mt, P)],
                        rhs=outT2[:, ts(nq, 512)],
                        start=False,
                        stop=True,
                    )
                    ysb = rlpool.tile([P, 512], F16, name="ysb", tag="ysb", bufs=4)
                    if mt % 2 == 0:
                        nc.scalar.copy(out=ysb, in_=yps)
                    else:
                        nc.vector.tensor_copy(out=ysb, in_=yps)
                    nc.sync.dma_start(out=yT[ts(mt, P), ts(nq, 512)], in_=ysb)
    return nc


_CACHE = threading.Lock(), {}


def _get_nc():
    lock, cache = _CACHE
    with lock:
        if "nc" not in cache:
            nc = bacc.Bacc("TRN2", target_bir_lowering=False, debug=False)
            _build_kernel(nc)
            nc.compile()
            cache["nc"] = nc
        return cache["nc"]


def _shard_inputs(x, w_qkv, b_qkv, w_proj):
    """Build the 8 per-core input maps (host-side sharding/layout)."""
    in_maps = []
    for c in range(NCORES):
        b = c // 4
        hg = c % 4
        h0 = HPC * hg
        qc = [np.s_[D * (h0 + i) : D * (h0 + i + 1)] for i in range(HPC)]
        kc = [np.s_[EMBED + D * (h0 + i) : EMBED + D * (h0 + i + 1)] for i in range(HPC)]
        vc = [np.s_[2 * EMBED + D * (h0 + i) : 2 * EMBED + D * (h0 + i + 1)] for i in range(HPC)]

        # projected w tiles: [q0|q1], [k0|k1], [q2|k2]; bias vector is laid
        # out per DEVICE qkT tile t=0..4 = [q0|q1],[k0|0],[0|k1],[0|q2],[0|k2]
        w_qk = np.zeros((EMBED, 3 * P), dtype=np.float32)
        b_qk = np.zeros((NT * P,), dtype=np.float32)
        halves = [
            (0, 0, qc[0]), (0, 1, qc[1]),
            (1, 0, kc[0]), (1, 1, kc[1]),
            (2, 0, qc[2]), (2, 1, kc[2]),
        ]
        for t, half, cols in halves:
            w_qk[:, t * P + half * D : t * P + half * D + D] = w_qkv[:, cols]
        bias_halves = [
            (0, 0, qc[0]), (0, 1, qc[1]),
            (1, 0, kc[0]),
            (2, 1, kc[1]),
            (3, 1, qc[2]),
            (4, 1, kc[2]),
        ]
        for t, half, cols in bias_halves:
            b_qk[t * P + half * D : t * P + half * D + D] = b_qkv[cols]

        w_v = np.concatenate([w_qkv[:, s] for s in vc], axis=1)
        # w_proj rows for these heads; B half zero-padded to K=128
        w_p = np.zeros((2 * P, EMBED), dtype=np.float32)
        w_p[0:P] = w_proj[D * h0 : D * h0 + P]
        w_p[P : P + D] = w_proj[D * h0 + P : D * (h0 + HPC)]
        in_maps.append(
            {
                "xT": np.ascontiguousarray(x[b].T).astype(np.float16),
                "w_qk": w_qk.astype(np.float16),
                "b_qk": b_qk,
                "w_v": np.ascontiguousarray(w_v).astype(np.float16),
                "w_p": w_p.astype(np.float16),
            }
        )
    return in_maps


def kernel(x, w_qkv, b_qkv, w_proj, b_proj, _results_hook=None):
    x = np.asarray(x, dtype=np.float32)
    w_qkv = np.asarray(w_qkv, dtype=np.float32)
    b_qkv = np.asarray(b_qkv, dtype=np.float32)
    w_proj = np.asarray(w_proj, dtype=np.float32)
    b_proj = np.asarray(b_proj, dtype=np.float32)

    nc = _get_nc()
    in_maps = _shard_inputs(x, w_qkv, b_qkv, w_proj)
    res = run_bass_kernel_spmd(nc, in_maps, core_ids=list(range(NCORES)))
    if _results_hook is not None:
        _results_hook(res)

    # unshard: sum the 4 head-group partials per batch, add bias terms
    b_v = b_qkv[2 * EMBED :]
    bias_row = b_v @ w_proj + b_proj  # [768]
    out = np.empty((B, S, EMBED), dtype=np.float32)
    for b in range(B):
        acc = np.zeros((EMBED, S), dtype=np.float32)
        for hg in range(4):
            acc += res.results[4 * b + hg]["yT"].astype(np.float32)
        out[b] = acc.T + bias_row
    return out
